# revision 4
# baseline (speedup 1.0000x reference)
"""Trainium2 Bass kernel v3: sigmoid multi-head attention (16 heads, S=2048,
D=1024, P=64) + final linear, head-sharded across 8 NeuronCores (2 heads/core).

Reference semantics (raw reshape): out[h*128 + r, f] = sum_{u,p}
attn[h, 16r+u, p] * W_fin[u*64+p, f].  Core c (heads 2c, 2c+1) owns output
rows [256c, 256c+256).  Whole kernel runs in the u-major permuted s order
(position i = u*128 + r), applied on the host.

v3 design (ACT-bound, fp8 DoubleRow PE):
  * ScalarE (ACT) is the fundamental bottleneck: 8.4M sigmoids/core =
    64 x [128,1024] activations x ~1.04us = 66.4us.  Everything else is
    organized to keep ACT 100% busy on sigmoid: no DMA issue, no copies on
    the scalar queue.
  * Projections: fp8e4 DoubleRow matmuls (contraction 256/instr, 0.5
    cyc/row).  K/Q: out [64,512] halves of [p2,t]-oriented kt2/qt2 (bf16
    for the score matmuls).  V: x-stationary DoubleRow gives V directly in
    natural [t, (h,p)] orientation (v2n fp8) -- no PE/DMA transposes at all.
  * Scores: bf16, [t,128 x (h,s) 1024] psum tiles, 2 matmuls K=64 each,
    single-shot.  Sigmoid reads the 2-bank tile, writes fp8 into sc
    pair-tiles [t, (i2, h2, s512)] (two j-tiles per tile = DoubleRow pairs).
  * Attention: fp8 DoubleRow in TRANSPOSED orientation (walrus rejects
    DoubleRow dst partitions >= 64): at[b,h] [p 64, s 512] += sum over 8
    t-pair matmuls (lhsT = v2n pair [t,i2,p64], rhs = sc pair [t,i2,s512],
    N=512 cost 256 cyc).  Each region is exactly one psum bank row (2KB)
    so start=True needs no seeding.  Blocks run SEQUENTIALLY so only 2
    at-banks are live.
  * Final linear: bf16, baseline structure.  ab = at-psum drained to bf16
    [64,512] = attnT[h][p, (t4, r)]; remap DMAs build ft chunks
    [k=(u2,p), (q,h,r')] directly (partition offset u2*64 via DMA) -- the
    transposed attn orientation makes the old PE transposes unnecessary.
    psf accumulation in 3 phases (after block 1, block 2, tail).
  * PSUM: ps 2x[128,1024] (4 banks) | at 2x[64,512] (2) | pj 2x[128,512]
    (2, K/Q proj psum + V landing pads); pj released mid-stream for psf (2).
  * Attention emission lags the sigmoid stream (block 0 at iters 11+P,
    block 1 at 19+2P) so all projection work fits the block-0 PE window.
  * ft chunks live in ONE [128, 2048] tile; the host s-permutation is
    block-major (u2, g, q, r') so each block's remap is 4 three-dim DMAs.
  * PE p-state: 8 dummy matmuls at t=0 warm the clock past the 3us ramp.
"""

import os

os.environ.setdefault("BASS_NEVER_TRACE", "1")

import numpy as np
from contextlib import ExitStack

import jax
import concourse.bacc as bacc
import concourse.bass as bass
import concourse.mybir as mybir
import concourse.tile as tile
from concourse.bass2jax import (
    _bass_exec_p,
    install_neuronx_cc_hook,
    partition_id_tensor,
)
from jax.experimental.shard_map import shard_map
from jax.sharding import Mesh, NamedSharding, PartitionSpec

S, D, H, P, F = 2048, 1024, 16, 64, 1024
NCORES = 8
HL = H // NCORES          # heads per core = 2
P2 = HL * P               # stacked head dim = 128
NB = S // 512             # 4 s-blocks (permuted order)
NT = S // 128             # 16 t-tiles
NC_F = 8                  # final-linear contraction chunks of 128

FP32 = mybir.dt.float32
BF16 = mybir.dt.bfloat16
FP8 = mybir.dt.float8e4
DR = mybir.MatmulPerfMode.DoubleRow
SIGMOID = mybir.ActivationFunctionType.Sigmoid
ADD = mybir.AluOpType.add


def build_kernel(ctx: ExitStack, tc: tile.TileContext, xt_d, x8_d, wq_d,
                 wk_d, wv_d, wf_d, out_d, dbg=None):
    nc = tc.nc

    w_pool = ctx.enter_context(tc.tile_pool(name="wts", bufs=1))
    qk_pool = ctx.enter_context(tc.tile_pool(name="qk", bufs=1))
    xt_pool = ctx.enter_context(tc.tile_pool(name="xt", bufs=1))
    sc_pool = ctx.enter_context(tc.tile_pool(name="sc", bufs=8))
    ab_pool = ctx.enter_context(tc.tile_pool(name="ab", bufs=2))
    fa_pool = ctx.enter_context(tc.tile_pool(name="fa", bufs=1))
    ot_pool = ctx.enter_context(tc.tile_pool(name="ot", bufs=1))

    # ---- PSUM: ps (4 banks) | at (2) | pj (2, later psf) ----
    ps_pool = ctx.enter_context(tc.tile_pool(name="ps", bufs=2, space="PSUM"))
    at_pool = ctx.enter_context(tc.tile_pool(name="at", bufs=2, space="PSUM"))
    pj_pool = tc.alloc_tile_pool(name="pj", bufs=2, space="PSUM", side="right")

    # sigmoid act-table preload before anything else on ACT
    warmz = qk_pool.tile([1, 128], BF16, tag="warmz")
    nc.vector.memset(warmz, 0.0)
    warmt = qk_pool.tile([1, 128], BF16, tag="warmt")
    nc.scalar.activation(warmt, warmz, SIGMOID)

    # ---------------- input DMAs (sync / gpsimd / vector lanes) ------
    FP32R = mybir.dt.float32r
    wk = w_pool.tile([128, D], FP32R, tag="wk")
    nc.gpsimd.dma_start(wk, wk_d.bitcast(FP32R))
    wq = w_pool.tile([128, D], FP32R, tag="wq")
    wv = w_pool.tile([128, 1024], FP8, tag="wv")
    nc.gpsimd.dma_start(wv, wv_d)

    # fp8 x tiles for the V projection (x-stationary DoubleRow)
    x8r = [None] * NB

    def emit_x8(b, engine):
        t8 = xt_pool.tile([128, 4096], FP8, tag=f"x8_{b}", name=f"x8_{b}")
        engine.dma_start(t8, x8_d[:, b * 4096:(b + 1) * 4096])
        x8r[b] = t8.rearrange("p (c i s) -> p c i s", c=4, i=2)

    def xslice(b, c, i, s0, s1):
        return x8r[b][:, c, :, s0:s1]

    # fp32 x chunk tiles [128,512] for K/Q (fp32r)
    DCH = D // 128
    xts = [[None] * DCH for _ in range(NB)]

    def emit_x(sb, ds, engine):
        for d in ds:
            t = xt_pool.tile([128, 512], FP32R, tag=f"x{sb}_{d}",
                             name=f"xt{sb}_{d}")
            engine.dma_start(
                t, xt_d[d * 128:(d + 1) * 128,
                        sb * 512:(sb + 1) * 512].bitcast(FP32R))
            xts[sb][d] = t

    # prologue-critical loads; later blocks are emitted from sched slots so
    # the sync queue stays interleaved with deadline order
    emit_x(0, range(0, 2), nc.sync)
    nc.sync.dma_start(wq, wq_d.bitcast(FP32R))
    emit_x(0, range(2, 6), nc.sync)
    emit_x(0, range(6, 8), nc.gpsimd)
    emit_x8(0, nc.scalar)               # ACT idle pre-stream; issue cost only
    emit_x(1, range(0, 4), nc.sync)
    emit_x(1, range(4, 8), nc.gpsimd)

    wf = w_pool.tile([128, NC_F * F], BF16, tag="wf")

    wv3 = wv.rearrange("p (c i m) -> p c i m", c=4, i=2)

    kt2 = qk_pool.tile([128, S], BF16, tag="kt2")   # [p2, t]
    qt2 = qk_pool.tile([128, S], BF16, tag="qt2")   # [p2, s-perm]
    v2n = qk_pool.tile([128, S], FP8, tag="v2n")    # [t_in, (j, h, p)]
    # all 8 final-linear contraction chunks in one tile [ (u2,p), (c,q,h,r') ]
    ft_all = qk_pool.tile([128, NC_F * 256], BF16, tag="ft_all")

    # ---------------- proj pieces ----------------
    pj_live = {}

    def piece_kq(w, dst, b, wname, half=None, copy_eng=None):
        """fp32r K/Q projection: half 0 = chunks 0-3 (allocates pj slot),
        half 1 = chunks 4-7 + psum->SBUF copy."""
        cols = slice(b * 512, (b + 1) * 512)

        def f():
            if half in (0, None):
                pj_live["pj"] = pj_pool.tile([128, 512], FP32, tag="pj",
                                             name=f"pj_{wname}{b}")
            pj = pj_live["pj"]
            ds = {0: range(0, 4), 1: range(4, 8), None: range(8)}[half]
            for d in ds:
                nc.tensor.matmul(pj, w[:, d * 128:(d + 1) * 128], xts[b][d],
                                 start=(d == 0), stop=(d == DCH - 1))
            if half in (1, None):
                (copy_eng or nc.vector).tensor_copy(dst[:, cols], pj)
        return f

    def piece_v(tb, half):
        """V proj for t-chunks [4*half, 4*half+4) of t-block tb, directly in
        natural [t, (h,p)] orientation (x stationary)."""
        def f():
            pv = pj_pool.tile([128, 512], FP32, tag="pj",
                              name=f"pv{tb}_{half}")
            for tc4 in range(4):
                tc = half * 4 + tc4
                for c in range(4):
                    nc.tensor.matmul(
                        pv[0:64, tc4 * 128:(tc4 + 1) * 128],
                        xslice(tb, c, None, tc * 64, (tc + 1) * 64),
                        wv3[:, c, :, :],
                        start=(c == 0), stop=(c == 3), perf_mode=DR)
            for tc4 in range(4):
                tc = half * 4 + tc4
                j = 4 * tb + tc // 2
                ph = tc % 2
                nc.vector.tensor_copy(
                    v2n[ph * 64:(ph + 1) * 64, j * 128:(j + 1) * 128],
                    pv[0:64, tc4 * 128:(tc4 + 1) * 128])
        return f

    # ---------------- stream state ----------------
    ps_tiles = {}
    sc_tiles = {}
    at_tiles = {}
    ab_tiles = {}
    fa_tiles = {}
    psf_holder = {}

    def emit_scores(i):
        b, j = i // NT, i % NT
        ps = ps_pool.tile([128, 1024], FP32, tag="ps", name=f"ps{b}_{j}")
        t0, s0 = j * 128, b * 512
        nc.tensor.matmul(ps[:, 0:512], kt2[0:64, t0:t0 + 128],
                         qt2[0:64, s0:s0 + 512])
        nc.tensor.matmul(ps[:, 512:1024], kt2[64:128, t0:t0 + 128],
                         qt2[64:128, s0:s0 + 512])
        ps_tiles[i] = ps

    def emit_sig(i):
        b, j = i // NT, i % NT
        pp = j // 2
        if j % 2 == 0:
            scp = sc_pool.tile([128, 2048], FP8, tag="sc", name=f"sc{b}_{pp}")
            sc_tiles[(b, pp)] = scp
        scp = sc_tiles[(b, pp)]
        nc.scalar.activation(scp[:, (j % 2) * 1024:(j % 2 + 1) * 1024],
                             ps_tiles[i], SIGMOID, scale=1.0 / P)

    def emit_attn(b, pp):
        scp = sc_tiles[(b, pp)].rearrange("t (i h s) -> t i h s", i=2, h=2)
        vsl = v2n[:, 2 * pp * 128:(2 * pp + 2) * 128].rearrange(
            "t (i x) -> t i x", i=2)
        for h in range(2):
            if pp == 0:
                at = at_pool.tile([64, 512], FP32, tag="at", name=f"at{b}{h}")
                at_tiles[(b, h)] = at
            nc.tensor.matmul(at_tiles[(b, h)],
                             vsl[:, :, h * 64:(h + 1) * 64],
                             scp[:, :, h, :],
                             start=(pp == 0), stop=(pp == NT // 2 - 1),
                             perf_mode=DR)

    def drain(b, h, act=False):
        def f():
            ab = ab_pool.tile([64, 512], BF16, tag="ab", name=f"ab{b}{h}")
            if act:
                nc.scalar.activation(ab, at_tiles[(b, h)],
                                     mybir.ActivationFunctionType.Copy)
            else:
                nc.vector.tensor_copy(ab, at_tiles[(b, h)])
            ab_tiles[(b, h)] = ab
        return f

    def remaps(b, engines=(None,), chunk_major=False):
        """Remap both heads' ab tiles into ft_all chunks 2b, 2b+1: one DMA
        per (h, u2) moving both chunks' columns at once (4 DMAs/block)."""
        def f():
            li = 0
            for h in range(2):
                for u2 in range(2):
                    # ab cols (u2, g, q, r'); tau = 4b + 2g + u2 -> chunk 2b+g
                    src = ab_tiles[(b, h)][:, u2 * 256:(u2 + 1) * 256
                                           ].rearrange("p (gq r) -> p gq r",
                                                       gq=4)
                    dst = ft_all[u2 * 64:(u2 + 1) * 64, :].rearrange(
                        "p (cq h2 r) -> p cq h2 r", cq=2 * NC_F,
                        h2=2)[:, 4 * b:4 * b + 4, h, :]
                    eng = engines[li % len(engines)]
                    li += 1
                    eng.dma_start(dst, src)
        return f

    def final_pieces(phase):
        """phase 0: chunks 0-3 -> fa copy; 1: chunks 4-5 -> fa add;
        2: chunks 6-7 -> ot = fa + psf, DMA out."""
        pieces = []
        state = {}
        cs = ((0, 1, 2, 3), (4, 5), (6, 7))[phase]

        def mk_mm(q, fc, sub):
            def f():
                psf_pool = psf_holder["pool"]
                if sub[0] == cs[0]:
                    state[(q, fc)] = psf_pool.tile(
                        [128, 512], FP32, tag="psf", name=f"psf{phase}{q}{fc}")
                psf = state[(q, fc)]
                for c in sub:
                    nc.tensor.matmul(
                        psf,
                        ft_all[:, c * 256 + q * 128: c * 256 + (q + 1) * 128],
                        wf[:, c * F + fc * 512: c * F + fc * 512 + 512],
                        start=(c == cs[0]), stop=(c == cs[-1]))
            return f

        def mk_tail(q, fc):
            def f():
                psf = state[(q, fc)]
                if phase == 0:
                    fa = fa_pool.tile([128, 512], FP32, tag=f"fa{q}{fc}",
                                      name=f"fa{q}{fc}")
                    nc.vector.tensor_copy(fa, psf)
                    fa_tiles[(q, fc)] = fa
                elif phase == 1:
                    nc.vector.tensor_tensor(fa_tiles[(q, fc)],
                                            fa_tiles[(q, fc)], psf, ADD)
                else:
                    ot = ot_pool.tile([128, 512], BF16, tag=f"ot{q}{fc}",
                                      name=f"ot{q}{fc}")
                    nc.vector.tensor_tensor(ot, fa_tiles[(q, fc)], psf, ADD)
                    eng = (nc.sync, nc.gpsimd, nc.scalar)[(q * 2 + fc) % 3]
                    eng.dma_start(
                        out_d[:, (q * 2 + fc) * 512:(q * 2 + fc + 1) * 512],
                        ot)
            return f

        for q in range(2):
            for fc in range(2):
                if phase == 0:
                    pieces.append(mk_mm(q, fc, (0, 1)))
                    pieces.append(mk_mm(q, fc, (2, 3)))
                else:
                    pieces.append(mk_mm(q, fc, cs))
                pieces.append(mk_tail(q, fc))
        return pieces

    # ---------------- prologue ----------------
    # PE clock warm-up: keep the PE busy from ~t=0 so the 3us p-state ramp
    # completes before the real projections run.
    zl = qk_pool.tile([1, 128], BF16, tag="zl")
    nc.vector.memset(zl, 0.0)
    zr = qk_pool.tile([1, 512], BF16, tag="zr")
    nc.vector.memset(zr, 0.0)
    pwarm = ps_pool.tile([128, 1024], FP32, tag="ps", name="pwarm")
    for w_i in range(8):
        nc.tensor.matmul(pwarm[:, 0:512], zl, zr, start=True, stop=True)
    # K0 and Q0 d-interleaved into two pj tiles: both finish right after the
    # last x0 chunk lands.
    pk0 = pj_pool.tile([128, 512], FP32, tag="pj", name="pk0")
    pq0 = pj_pool.tile([128, 512], FP32, tag="pj", name="pq0")
    for d in range(DCH):
        nc.tensor.matmul(pk0, wk[:, d * 128:(d + 1) * 128], xts[0][d],
                         start=(d == 0), stop=(d == DCH - 1))
        nc.tensor.matmul(pq0, wq[:, d * 128:(d + 1) * 128], xts[0][d],
                         start=(d == 0), stop=(d == DCH - 1))
    nc.scalar.activation(kt2[:, 0:512], pk0,
                         mybir.ActivationFunctionType.Copy)
    nc.vector.tensor_copy(qt2[:, 0:512], pq0)
    emit_scores(0)
    emit_scores(1)

    # ---------------- slack schedule ----------------
    sched = {i: [] for i in range(NB * NT + 4)}
    def dma_piece(f):
        return f

    sched[0].append(lambda: emit_x8(1, nc.sync))
    sched[0].append(piece_v(0, 0))
    sched[1].append(lambda: emit_x(2, range(0, 4), nc.sync))
    sched[1].append(lambda: emit_x(2, range(4, 8), nc.gpsimd))
    sched[1].append(piece_kq(wk, kt2, 1, "k", 0))
    sched[2].append(piece_kq(wk, kt2, 1, "k", 1))
    sched[3].append(lambda: emit_x8(2, nc.sync))
    sched[3].append(piece_v(0, 1))
    sched[4].append(lambda: emit_x(3, range(0, 4), nc.sync))
    sched[4].append(lambda: emit_x(3, range(4, 8), nc.gpsimd))
    sched[4].append(piece_v(1, 0))
    sched[5].append(piece_kq(wk, kt2, 2, "k", 0))
    sched[6].append(piece_kq(wk, kt2, 2, "k", 1))
    sched[7].append(piece_v(1, 1))
    sched[7].append(lambda: emit_x8(3, nc.sync))
    sched[8].append(piece_v(2, 0))
    sched[9].append(piece_kq(wk, kt2, 3, "k", 0))
    sched[10].append(piece_kq(wk, kt2, 3, "k", 1))
    sched[10].append(lambda: [nc.sync.dma_start(
        wf[:, c * F:(c + 1) * F], wf_d[:, c * F:(c + 1) * F])
        for c in range(NC_F)])
    sched[11].append(piece_v(2, 1))
    sched[12].append(piece_v(3, 0))
    sched[13].append(piece_kq(wq, qt2, 1, "q", 0))
    sched[14].append(piece_kq(wq, qt2, 1, "q", 1))
    sched[15].append(piece_v(3, 1))
    sched[17].append(piece_kq(wq, qt2, 2, "q", 0))
    sched[18].append(piece_kq(wq, qt2, 2, "q", 1))
    sched[19].append(piece_kq(wq, qt2, 3, "q", 0))
    sched[20].append(piece_kq(wq, qt2, 3, "q", 1))

    def open_psf():
        pj_pool.release()
        cm = tc.tile_pool(name="psf", bufs=2, space="PSUM")
        psf_holder["cm"] = cm
        psf_holder["pool"] = cm.__enter__()
    sched[23].append(open_psf)

    # attention emission iters: block 0 and 1 lag so their V/proj pressure
    # leaves the block-0 crunch window; blocks 2,3 run at minimum lag.
    attn_iter = {}
    for pp in range(8):
        attn_iter[(0, pp)] = 11 + pp
        attn_iter[(1, pp)] = 19 + 2 * pp
        attn_iter[(2, pp)] = 32 + 2 * pp + 2
        attn_iter[(3, pp)] = 48 + 2 * pp + 2
    attn_at = {}
    for (bb, pp), it in attn_iter.items():
        attn_at.setdefault(it, []).append((bb, pp))

    # drains and remaps at block boundaries (sched runs after attn_at,
    # so same-iter drain follows that block's last attn)
    for b, (di, ri) in enumerate(((18, 19), (33, 34), (48, 49))):
        sched[di].append(drain(b, 0))
        sched[di].append(drain(b, 1))
        sched[ri].append(remaps(b, (nc.sync, nc.gpsimd)))

    # final phases: A after block 1 drains, B after block 2 drains
    for k, p in enumerate(final_pieces(0)):
        sched[35 + k].append(p)
    for k, p in enumerate(final_pieces(1)):
        sched[51 + k].append(p)

    # ---------------- main stream ----------------
    for i in range(NB * NT):
        for bb, pp in attn_at.get(i, ()):
            emit_attn(bb, pp)
        for p in sched[i]:
            p()
        if i + 2 < NB * NT:
            emit_scores(i + 2)
        emit_sig(i)

    # ---------------- tail ----------------
    for it in sorted(k for k in attn_at if k >= NB * NT):
        for bb, pp in attn_at[it]:
            emit_attn(bb, pp)
    drain(NB - 1, 0)()
    drain(NB - 1, 1, act=True)()
    remaps(NB - 1, (nc.sync, nc.gpsimd), chunk_major=True)()
    for p in final_pieces(2):
        p()
    psf_holder["cm"].__exit__(None, None, None)

    if dbg is not None:
        nc.sync.dma_start(dbg["kt2"], kt2)
        nc.sync.dma_start(dbg["qt2"], qt2)
        nc.sync.dma_start(dbg["v2n"], v2n)
        nc.sync.dma_start(dbg["sc00"], sc_tiles[(0, 0)])
        nc.sync.dma_start(dbg["ab00"], ab_tiles[(0, 0)])
        for c in range(NC_F):
            nc.sync.dma_start(dbg[f"ft{c}"], ft_all[:, c * 256:(c + 1) * 256])


def build_bass(replicas: int = 1) -> bass.Bass:
    nc = bacc.Bacc("TRN2", target_bir_lowering=False, debug=False,
                   num_devices=NCORES)
    xt_d = nc.dram_tensor("xt", [D, S], FP32, kind="ExternalInput").ap()
    x8_d = nc.dram_tensor("x8", [128, NB * 4096], FP8,
                          kind="ExternalInput").ap()
    wq_d = nc.dram_tensor("wq", [128, D], FP32, kind="ExternalInput").ap()
    wk_d = nc.dram_tensor("wk", [128, D], FP32, kind="ExternalInput").ap()
    wv_d = nc.dram_tensor("wv", [128, 1024], FP8, kind="ExternalInput").ap()
    wf_d = nc.dram_tensor("wf", [128, NC_F * F], BF16,
                          kind="ExternalInput").ap()
    # compact layout: [ (h,r'), (q,fc)*512 + f' ] -- host unscatters rows
    out_d = nc.dram_tensor("out", [128, 4 * 512], BF16,
                           kind="ExternalOutput").ap()
    dbg = None
    if os.environ.get("V3_DEBUG"):
        dbg = {
            "kt2": nc.dram_tensor("dbg_kt2", [128, S], BF16,
                                  kind="ExternalOutput").ap(),
            "qt2": nc.dram_tensor("dbg_qt2", [128, S], BF16,
                                  kind="ExternalOutput").ap(),
            "v2n": nc.dram_tensor("dbg_v2n", [128, S], FP8,
                                  kind="ExternalOutput").ap(),
            "sc00": nc.dram_tensor("dbg_sc00", [128, 2048], FP8,
                                   kind="ExternalOutput").ap(),
            "ab00": nc.dram_tensor("dbg_ab00", [64, 512], BF16,
                                   kind="ExternalOutput").ap(),
        }
        for c in range(NC_F):
            dbg[f"ft{c}"] = nc.dram_tensor(f"dbg_ft{c}", [128, 256], BF16,
                                           kind="ExternalOutput").ap()
    with tile.TileContext(nc) as tc:
        for _ in range(replicas):
            with ExitStack() as ctx:
                build_kernel(ctx, tc, xt_d, x8_d, wq_d, wk_d, wv_d, wf_d,
                             out_d, dbg)
    nc.finalize()
    return nc


_NC_CACHE = None
_EXEC_CACHE = None


def _get_nc():
    global _NC_CACHE
    if _NC_CACHE is None:
        _NC_CACHE = build_bass()
    return _NC_CACHE


def _get_executor():
    global _EXEC_CACHE
    if _EXEC_CACHE is not None:
        return _EXEC_CACHE
    import concourse.mybir as mybir

    nc = _get_nc()
    install_neuronx_cc_hook()
    partition_name = (nc.partition_id_tensor.name
                      if nc.partition_id_tensor else None)
    in_names, out_names, out_avals = [], [], []
    out_shapes = []
    for alloc in nc.m.functions[0].allocations:
        if not isinstance(alloc, mybir.MemoryLocationSet):
            continue
        name = alloc.memorylocations[0].name
        if alloc.kind == "ExternalInput":
            if name != partition_name:
                in_names.append(name)
        elif alloc.kind == "ExternalOutput":
            shape = tuple(alloc.tensor_shape)
            dtype = mybir.dt.np(alloc.dtype)
            out_names.append(name)
            out_avals.append(jax.core.ShapedArray(shape, dtype))
            out_shapes.append((shape, dtype))
    n_params = len(in_names)
    all_names = list(in_names) + list(out_names)
    if partition_name is not None:
        all_names.append(partition_name)

    def _body(*args):
        operands = list(args)
        if partition_name is not None:
            operands.append(partition_id_tensor())
        outs = _bass_exec_p.bind(
            *operands,
            out_avals=tuple(out_avals),
            in_names=tuple(all_names),
            out_names=tuple(out_names),
            lowering_input_output_aliases=(),
            sim_require_finite=True,
            sim_require_nnan=True,
            nc=nc,
        )
        return tuple(outs)

    devices = jax.devices()[:NCORES]
    mesh = Mesh(np.asarray(devices), ("core",))
    n_args = n_params + len(out_names)
    sharded = jax.jit(shard_map(
        _body, mesh=mesh,
        in_specs=(PartitionSpec("core"),) * n_args,
        out_specs=(PartitionSpec("core"),) * len(out_names),
        check_rep=False))
    _EXEC_CACHE = (sharded, mesh, in_names, out_names, out_shapes)
    return _EXEC_CACHE


def _run_spmd(in_maps):
    sharded, mesh, in_names, out_names, out_shapes = _get_executor()
    sh = NamedSharding(mesh, PartitionSpec("core"))
    args = [np.concatenate([im[name] for im in in_maps], axis=0)
            for name in in_names]
    for shape, dtype in out_shapes:
        args.append(np.zeros((NCORES * shape[0],) + shape[1:], dtype))
    dev_args = [jax.device_put(a, sh) for a in args]
    outs = sharded(*dev_args)
    jax.block_until_ready(outs)
    results = []
    for c in range(NCORES):
        res = {}
        for i, name in enumerate(out_names):
            g = np.asarray(outs[i])
            d0 = g.shape[0] // NCORES
            res[name] = g[c * d0:(c + 1) * d0]
        results.append(res)
    return results


def _layout_w32(w, c):
    """[H, D, P] -> per-core [128, (dchunk8, p2)] fp32 (baseline fp32r)."""
    wl = np.transpose(w[HL * c:HL * (c + 1)], (1, 0, 2)).reshape(D, P2)
    wl = wl.reshape(8, 128, P2).transpose(1, 0, 2).reshape(128, 8 * P2)
    return np.ascontiguousarray(wl, dtype=np.float32)


def _layout_w8(w, c):
    """[H, D, P] -> per-core [128, (c4, i2, m128)] fp8 (DoubleRow pairs)."""
    import ml_dtypes
    wl = np.transpose(w[HL * c:HL * (c + 1)], (1, 0, 2)).reshape(D, P2)
    wl = wl.reshape(4, 2, 128, P2).transpose(2, 0, 1, 3).reshape(128, 1024)
    return np.ascontiguousarray(wl).astype(ml_dtypes.float8_e4m3)


def make_in_maps(x, Qw, Kw, Vw, W_fin):
    import ml_dtypes
    x = np.asarray(x, dtype=np.float32)
    Qw = np.asarray(Qw, dtype=np.float32)
    Kw = np.asarray(Kw, dtype=np.float32)
    Vw = np.asarray(Vw, dtype=np.float32)
    W_fin = np.asarray(W_fin, dtype=np.float32)

    # block-major permutation: pos = b*512 + u2*256 + g*128 + r with
    # u = 4b + 2g + u2, s_orig = 16r + u  (keeps final-linear chunks
    # contiguous per (u2, g) for 3D-balanced remap DMAs)
    pos = np.arange(S)
    b_i, u2_i = pos // 512, (pos // 256) % 2
    g_i, r_i = (pos // 128) % 2, pos % 128
    s_orig = 16 * r_i + 4 * b_i + 2 * g_i + u2_i
    xt = np.ascontiguousarray(x.T)                      # [D, S]
    xtp = np.ascontiguousarray(xt[:, s_orig])
    # fp8 copy in [p, (b, c, i, s)] with d = c*256 + i*128 + p
    xf = xtp.reshape(4, 2, 128, NB, 512).transpose(2, 3, 0, 1, 4).reshape(
        128, NB * 4096)
    xf = np.ascontiguousarray(xf).astype(ml_dtypes.float8_e4m3)
    # wf: natural W_fin contraction chunks [128, c*F + f], bf16
    wf = np.ascontiguousarray(
        W_fin.reshape(NC_F, 128, F).transpose(1, 0, 2).reshape(128, NC_F * F)
    ).astype(ml_dtypes.bfloat16)

    in_maps = []
    for c in range(NCORES):
        in_maps.append({
            "xt": xtp,
            "x8": xf,
            "wq": _layout_w32(Qw, c),
            "wk": _layout_w32(Kw, c),
            "wv": _layout_w8(Vw, c),
            "wf": wf,
        })
    return in_maps


def assemble_out(results, b_fin):
    b_fin = np.asarray(b_fin, dtype=np.float32)
    cores = []
    for c in range(NCORES):
        buf = results[c]["out"].astype(np.float32)      # [128, 2048]
        v = buf.reshape(2, 64, 2, 2, 512)               # [h, r', q, fc, f]
        cores.append(v.transpose(0, 2, 1, 3, 4).reshape(256, F))
    out = np.concatenate(cores, axis=0)
    return (out + b_fin).astype(np.float32)


def kernel(x, Qw, Kw, Vw, W_fin, b_fin):
    in_maps = make_in_maps(x, Qw, Kw, Vw, W_fin)
    results = _run_spmd(in_maps)
    return assemble_out(results, b_fin)


# revision 6
# speedup vs baseline: 1.0167x; 1.0167x over previous
"""Trainium2 Bass kernel v3: sigmoid multi-head attention (16 heads, S=2048,
D=1024, P=64) + final linear, head-sharded across 8 NeuronCores (2 heads/core).

Reference semantics (raw reshape): out[h*128 + r, f] = sum_{u,p}
attn[h, 16r+u, p] * W_fin[u*64+p, f].  Core c (heads 2c, 2c+1) owns output
rows [256c, 256c+256).  Whole kernel runs in the u-major permuted s order
(position i = u*128 + r), applied on the host.

v3 design (ACT-bound, fp8 DoubleRow PE):
  * ScalarE (ACT) is the fundamental bottleneck: 8.4M sigmoids/core =
    64 x [128,1024] activations x ~1.04us = 66.4us.  Everything else is
    organized to keep ACT 100% busy on sigmoid: no DMA issue, no copies on
    the scalar queue.
  * Projections: fp8e4 DoubleRow matmuls (contraction 256/instr, 0.5
    cyc/row).  K/Q: out [64,512] halves of [p2,t]-oriented kt2/qt2 (bf16
    for the score matmuls).  V: x-stationary DoubleRow gives V directly in
    natural [t, (h,p)] orientation (v2n fp8) -- no PE/DMA transposes at all.
  * Scores: bf16, [t,128 x (h,s) 1024] psum tiles, 2 matmuls K=64 each,
    single-shot.  Sigmoid reads the 2-bank tile, writes fp8 into sc
    pair-tiles [t, (i2, h2, s512)] (two j-tiles per tile = DoubleRow pairs).
  * Attention: fp8 DoubleRow in TRANSPOSED orientation (walrus rejects
    DoubleRow dst partitions >= 64): at[b,h] [p 64, s 512] += sum over 8
    t-pair matmuls (lhsT = v2n pair [t,i2,p64], rhs = sc pair [t,i2,s512],
    N=512 cost 256 cyc).  Each region is exactly one psum bank row (2KB)
    so start=True needs no seeding.  Blocks run SEQUENTIALLY so only 2
    at-banks are live.
  * Final linear: bf16, baseline structure.  ab = at-psum drained to bf16
    [64,512] = attnT[h][p, (t4, r)]; remap DMAs build ft chunks
    [k=(u2,p), (q,h,r')] directly (partition offset u2*64 via DMA) -- the
    transposed attn orientation makes the old PE transposes unnecessary.
    psf accumulation in 3 phases (after block 1, block 2, tail).
  * PSUM: ps 2x[128,1024] (4 banks) | at 2x[64,512] (2) | pj 2x[128,512]
    (2, K/Q proj psum + V landing pads); pj released mid-stream for psf (2).
  * Attention emission lags the sigmoid stream (block 0 at iters 11+P,
    block 1 at 19+2P) so all projection work fits the block-0 PE window.
  * ft chunks live in ONE [128, 2048] tile; the host s-permutation is
    block-major (u2, g, q, r') so each block's remap is 4 three-dim DMAs.
  * PE p-state: 8 dummy matmuls at t=0 warm the clock past the 3us ramp.
"""

import os

os.environ.setdefault("BASS_NEVER_TRACE", "1")

import numpy as np
from contextlib import ExitStack

import jax
import concourse.bacc as bacc
import concourse.bass as bass
import concourse.mybir as mybir
import concourse.tile as tile
from concourse.bass2jax import (
    _bass_exec_p,
    install_neuronx_cc_hook,
    partition_id_tensor,
)
from jax.experimental.shard_map import shard_map
from jax.sharding import Mesh, NamedSharding, PartitionSpec

S, D, H, P, F = 2048, 1024, 16, 64, 1024
NCORES = 8
HL = H // NCORES          # heads per core = 2
P2 = HL * P               # stacked head dim = 128
NB = S // 512             # 4 s-blocks (permuted order)
NT = S // 128             # 16 t-tiles
NC_F = 8                  # final-linear contraction chunks of 128

FP32 = mybir.dt.float32
BF16 = mybir.dt.bfloat16
FP8 = mybir.dt.float8e4
DR = mybir.MatmulPerfMode.DoubleRow
SIGMOID = mybir.ActivationFunctionType.Sigmoid
ADD = mybir.AluOpType.add


def build_kernel(ctx: ExitStack, tc: tile.TileContext, xt_d, x8_d, wq_d,
                 wk_d, wv_d, wf_d, out_d, dbg=None):
    nc = tc.nc

    w_pool = ctx.enter_context(tc.tile_pool(name="wts", bufs=1))
    qk_pool = ctx.enter_context(tc.tile_pool(name="qk", bufs=1))
    xt_pool = ctx.enter_context(tc.tile_pool(name="xt", bufs=1))
    sc_pool = ctx.enter_context(tc.tile_pool(name="sc", bufs=8))
    ab_pool = ctx.enter_context(tc.tile_pool(name="ab", bufs=2))
    fa_pool = ctx.enter_context(tc.tile_pool(name="fa", bufs=1))
    ot_pool = ctx.enter_context(tc.tile_pool(name="ot", bufs=1))

    # ---- PSUM: ps (4 banks) | at (2) | pj (2, later psf) ----
    ps_pool = ctx.enter_context(tc.tile_pool(name="ps", bufs=2, space="PSUM"))
    at_pool = ctx.enter_context(tc.tile_pool(name="at", bufs=2, space="PSUM"))
    pj_pool = tc.alloc_tile_pool(name="pj", bufs=2, space="PSUM", side="right")

    # sigmoid act-table preload before anything else on ACT
    warmz = qk_pool.tile([1, 128], BF16, tag="warmz")
    nc.vector.memset(warmz, 0.0)
    warmt = qk_pool.tile([1, 128], BF16, tag="warmt")
    nc.scalar.activation(warmt, warmz, SIGMOID)

    # ---------------- input DMAs (sync / gpsimd / vector lanes) ------
    FP32R = mybir.dt.float32r
    wk = w_pool.tile([128, D], FP32R, tag="wk")
    nc.gpsimd.dma_start(wk, wk_d.bitcast(FP32R))
    wq = w_pool.tile([128, D], FP32R, tag="wq")
    wv = w_pool.tile([128, 1024], FP8, tag="wv")
    nc.gpsimd.dma_start(wv, wv_d)

    # fp8 x tiles for the V projection (x-stationary DoubleRow)
    x8r = [None] * NB

    def emit_x8(b, engine):
        t8 = xt_pool.tile([128, 4096], FP8, tag=f"x8_{b}", name=f"x8_{b}")
        engine.dma_start(t8, x8_d[:, b * 4096:(b + 1) * 4096])
        x8r[b] = t8.rearrange("p (c i s) -> p c i s", c=4, i=2)

    def xslice(b, c, i, s0, s1):
        return x8r[b][:, c, :, s0:s1]

    # fp32 x chunk tiles [128,512] for K/Q (fp32r)
    DCH = D // 128
    xts = [[None] * DCH for _ in range(NB)]

    def emit_x(sb, ds, engine):
        for d in ds:
            t = xt_pool.tile([128, 512], FP32R, tag=f"x{sb}_{d}",
                             name=f"xt{sb}_{d}")
            engine.dma_start(
                t, xt_d[d * 128:(d + 1) * 128,
                        sb * 512:(sb + 1) * 512].bitcast(FP32R))
            xts[sb][d] = t

    # prologue-critical loads; later blocks are emitted from sched slots so
    # the sync queue stays interleaved with deadline order
    emit_x(0, range(0, 2), nc.sync)
    nc.sync.dma_start(wq, wq_d.bitcast(FP32R))
    emit_x(0, range(2, 6), nc.sync)
    emit_x(0, range(6, 8), nc.gpsimd)
    emit_x8(0, nc.scalar)               # ACT idle pre-stream; issue cost only
    emit_x(1, range(0, 4), nc.sync)
    emit_x(1, range(4, 8), nc.gpsimd)

    wf = w_pool.tile([128, NC_F * F], BF16, tag="wf")

    wv3 = wv.rearrange("p (c i m) -> p c i m", c=4, i=2)

    kt2 = qk_pool.tile([128, S], BF16, tag="kt2")   # [p2, t]
    qt2 = qk_pool.tile([128, S], BF16, tag="qt2")   # [p2, s-perm]
    v2n = qk_pool.tile([128, S], FP8, tag="v2n")    # [t_in, (j, h, p)]
    # all 8 final-linear contraction chunks in one tile [ (u2,p), (c,q,h,r') ]
    ft_all = qk_pool.tile([128, NC_F * 256], BF16, tag="ft_all")

    # ---------------- proj pieces ----------------
    pj_live = {}

    def piece_kq(w, dst, b, wname, half=None, copy_eng=None):
        """fp32r K/Q projection: half 0 = chunks 0-3 (allocates pj slot),
        half 1 = chunks 4-7 + psum->SBUF copy."""
        cols = slice(b * 512, (b + 1) * 512)

        def f():
            if half in (0, None):
                pj_live["pj"] = pj_pool.tile([128, 512], FP32, tag="pj",
                                             name=f"pj_{wname}{b}")
            pj = pj_live["pj"]
            ds = {0: range(0, 4), 1: range(4, 8), None: range(8)}[half]
            for d in ds:
                nc.tensor.matmul(pj, w[:, d * 128:(d + 1) * 128], xts[b][d],
                                 start=(d == 0), stop=(d == DCH - 1))
            if half in (1, None):
                (copy_eng or nc.vector).tensor_copy(dst[:, cols], pj)
        return f

    def piece_v(tb, half):
        """V proj for t-chunks [4*half, 4*half+4) of t-block tb, directly in
        natural [t, (h,p)] orientation (x stationary)."""
        def f():
            pv = pj_pool.tile([128, 512], FP32, tag="pj",
                              name=f"pv{tb}_{half}")
            for tc4 in range(4):
                tc = half * 4 + tc4
                for c in range(4):
                    nc.tensor.matmul(
                        pv[0:64, tc4 * 128:(tc4 + 1) * 128],
                        xslice(tb, c, None, tc * 64, (tc + 1) * 64),
                        wv3[:, c, :, :],
                        start=(c == 0), stop=(c == 3), perf_mode=DR)
            for tc4 in range(4):
                tc = half * 4 + tc4
                j = 4 * tb + tc // 2
                ph = tc % 2
                nc.vector.tensor_copy(
                    v2n[ph * 64:(ph + 1) * 64, j * 128:(j + 1) * 128],
                    pv[0:64, tc4 * 128:(tc4 + 1) * 128])
        return f

    # ---------------- stream state ----------------
    ps_tiles = {}
    sc_tiles = {}
    at_tiles = {}
    ab_tiles = {}
    fa_tiles = {}
    psf_holder = {}

    def emit_scores(i):
        b, j = i // NT, i % NT
        ps = ps_pool.tile([128, 1024], FP32, tag="ps", name=f"ps{b}_{j}")
        t0, s0 = j * 128, b * 512
        nc.tensor.matmul(ps[:, 0:512], kt2[0:64, t0:t0 + 128],
                         qt2[0:64, s0:s0 + 512])
        nc.tensor.matmul(ps[:, 512:1024], kt2[64:128, t0:t0 + 128],
                         qt2[64:128, s0:s0 + 512])
        ps_tiles[i] = ps

    def emit_sig(i):
        b, j = i // NT, i % NT
        pp = j // 2
        if j % 2 == 0:
            scp = sc_pool.tile([128, 2048], FP8, tag="sc", name=f"sc{b}_{pp}")
            sc_tiles[(b, pp)] = scp
        scp = sc_tiles[(b, pp)]
        nc.scalar.activation(scp[:, (j % 2) * 1024:(j % 2 + 1) * 1024],
                             ps_tiles[i], SIGMOID, scale=1.0 / P)

    def emit_attn(b, pp):
        scp = sc_tiles[(b, pp)].rearrange("t (i h s) -> t i h s", i=2, h=2)
        vsl = v2n[:, 2 * pp * 128:(2 * pp + 2) * 128].rearrange(
            "t (i x) -> t i x", i=2)
        for h in range(2):
            if pp == 0:
                at = at_pool.tile([64, 512], FP32, tag="at", name=f"at{b}{h}")
                at_tiles[(b, h)] = at
            nc.tensor.matmul(at_tiles[(b, h)],
                             vsl[:, :, h * 64:(h + 1) * 64],
                             scp[:, :, h, :],
                             start=(pp == 0), stop=(pp == NT // 2 - 1),
                             perf_mode=DR)

    def drain(b, h, act=False):
        def f():
            ab = ab_pool.tile([64, 512], BF16, tag="ab", name=f"ab{b}{h}")
            if act:
                nc.scalar.activation(ab, at_tiles[(b, h)],
                                     mybir.ActivationFunctionType.Copy)
            else:
                nc.vector.tensor_copy(ab, at_tiles[(b, h)])
            ab_tiles[(b, h)] = ab
        return f

    def remaps(b, engines=(None,), chunk_major=False):
        """Remap both heads' ab tiles into ft_all chunks 2b, 2b+1: one DMA
        per (h, u2) moving both chunks' columns at once (4 DMAs/block)."""
        def f():
            li = 0
            for h in range(2):
                for u2 in range(2):
                    # ab cols (u2, g, q, r'); tau = 4b + 2g + u2 -> chunk 2b+g
                    src = ab_tiles[(b, h)][:, u2 * 256:(u2 + 1) * 256
                                           ].rearrange("p (gq r) -> p gq r",
                                                       gq=4)
                    dst = ft_all[u2 * 64:(u2 + 1) * 64, :].rearrange(
                        "p (cq h2 r) -> p cq h2 r", cq=2 * NC_F,
                        h2=2)[:, 4 * b:4 * b + 4, h, :]
                    eng = engines[li % len(engines)]
                    li += 1
                    eng.dma_start(dst, src)
        return f

    def final_pieces(phase):
        """phase 0: chunks 0-3 -> fa copy; 1: chunks 4-5 -> fa add;
        2: chunks 6-7 -> ot = fa + psf, DMA out."""
        pieces = []
        state = {}
        cs = ((0, 1, 2, 3), (4, 5), (6, 7))[phase]

        def mk_mm(q, fc, sub):
            def f():
                psf_pool = psf_holder["pool"]
                if sub[0] == cs[0]:
                    state[(q, fc)] = psf_pool.tile(
                        [128, 512], FP32, tag="psf", name=f"psf{phase}{q}{fc}")
                psf = state[(q, fc)]
                for c in sub:
                    nc.tensor.matmul(
                        psf,
                        ft_all[:, c * 256 + q * 128: c * 256 + (q + 1) * 128],
                        wf[:, c * F + fc * 512: c * F + fc * 512 + 512],
                        start=(c == cs[0]), stop=(c == cs[-1]))
            return f

        def mk_tail(q, fc):
            def f():
                psf = state[(q, fc)]
                if phase == 0:
                    fa = fa_pool.tile([128, 512], FP32, tag=f"fa{q}{fc}",
                                      name=f"fa{q}{fc}")
                    nc.vector.tensor_copy(fa, psf)
                    fa_tiles[(q, fc)] = fa
                elif phase == 1:
                    nc.vector.tensor_tensor(fa_tiles[(q, fc)],
                                            fa_tiles[(q, fc)], psf, ADD)
                else:
                    ot = ot_pool.tile([128, 512], BF16, tag=f"ot{q}{fc}",
                                      name=f"ot{q}{fc}")
                    nc.vector.tensor_tensor(ot, fa_tiles[(q, fc)], psf, ADD)
                    eng = (nc.sync, nc.gpsimd, nc.scalar)[(q * 2 + fc) % 3]
                    eng.dma_start(
                        out_d[:, (q * 2 + fc) * 512:(q * 2 + fc + 1) * 512],
                        ot)
            return f

        for q in range(2):
            for fc in range(2):
                if phase == 0:
                    pieces.append(mk_mm(q, fc, (0, 1)))
                    pieces.append(mk_mm(q, fc, (2, 3)))
                else:
                    pieces.append(mk_mm(q, fc, cs))
                pieces.append(mk_tail(q, fc))
        return pieces

    # ---------------- prologue ----------------
    # PE clock warm-up: keep the PE busy from ~t=0 so the 3us p-state ramp
    # completes before the real projections run.
    zl = qk_pool.tile([1, 128], BF16, tag="zl")
    nc.vector.memset(zl, 0.0)
    zr = qk_pool.tile([1, 512], BF16, tag="zr")
    nc.vector.memset(zr, 0.0)
    pwarm = ps_pool.tile([128, 1024], FP32, tag="ps", name="pwarm")
    for w_i in range(3):
        nc.tensor.matmul(pwarm[:, 0:512], zl, zr, start=True, stop=True)
    # K0 and Q0 d-interleaved into two pj tiles: both finish right after the
    # last x0 chunk lands.
    pk0 = pj_pool.tile([128, 512], FP32, tag="pj", name="pk0")
    pq0 = pj_pool.tile([128, 512], FP32, tag="pj", name="pq0")
    for d in range(DCH):
        nc.tensor.matmul(pk0, wk[:, d * 128:(d + 1) * 128], xts[0][d],
                         start=(d == 0), stop=(d == DCH - 1))
        nc.tensor.matmul(pq0, wq[:, d * 128:(d + 1) * 128], xts[0][d],
                         start=(d == 0), stop=(d == DCH - 1))
    nc.scalar.activation(kt2[:, 0:512], pk0,
                         mybir.ActivationFunctionType.Copy)
    nc.vector.tensor_copy(qt2[:, 0:512], pq0)
    emit_scores(0)
    emit_scores(1)

    # ---------------- slack schedule ----------------
    sched = {i: [] for i in range(NB * NT + 4)}
    def dma_piece(f):
        return f

    sched[0].append(lambda: emit_x8(1, nc.sync))
    sched[0].append(piece_v(0, 0))
    sched[1].append(lambda: emit_x(2, range(0, 4), nc.sync))
    sched[1].append(lambda: emit_x(2, range(4, 8), nc.gpsimd))
    sched[1].append(piece_kq(wk, kt2, 1, "k", 0))
    sched[2].append(piece_kq(wk, kt2, 1, "k", 1))
    sched[3].append(lambda: emit_x8(2, nc.sync))
    sched[3].append(piece_v(0, 1))
    sched[4].append(lambda: emit_x(3, range(0, 4), nc.sync))
    sched[4].append(lambda: emit_x(3, range(4, 8), nc.gpsimd))
    sched[5].append(piece_v(1, 0))
    sched[5].append(piece_kq(wk, kt2, 2, "k", 0))
    sched[6].append(piece_kq(wk, kt2, 2, "k", 1))
    sched[7].append(piece_v(1, 1))
    sched[7].append(lambda: emit_x8(3, nc.sync))
    sched[8].append(piece_v(2, 0))
    sched[9].append(piece_kq(wk, kt2, 3, "k", 0))
    sched[10].append(piece_kq(wk, kt2, 3, "k", 1))
    sched[10].append(lambda: [nc.sync.dma_start(
        wf[:, c * F:(c + 1) * F], wf_d[:, c * F:(c + 1) * F])
        for c in range(NC_F)])
    sched[11].append(piece_v(2, 1))
    sched[12].append(piece_v(3, 0))
    sched[13].append(piece_kq(wq, qt2, 1, "q", 0))
    sched[14].append(piece_kq(wq, qt2, 1, "q", 1))
    sched[15].append(piece_v(3, 1))
    sched[17].append(piece_kq(wq, qt2, 2, "q", 0))
    sched[18].append(piece_kq(wq, qt2, 2, "q", 1))
    sched[19].append(piece_kq(wq, qt2, 3, "q", 0))
    sched[20].append(piece_kq(wq, qt2, 3, "q", 1))

    def open_psf():
        pj_pool.release()
        cm = tc.tile_pool(name="psf", bufs=2, space="PSUM")
        psf_holder["cm"] = cm
        psf_holder["pool"] = cm.__enter__()
    sched[23].append(open_psf)

    # attention emission iters: block 0 and 1 lag so their V/proj pressure
    # leaves the block-0 crunch window; blocks 2,3 run at minimum lag.
    attn_iter = {}
    for pp in range(8):
        attn_iter[(0, pp)] = 11 + pp
        attn_iter[(1, pp)] = 19 + 2 * pp
        attn_iter[(2, pp)] = 32 + 2 * pp + 2
        attn_iter[(3, pp)] = 48 + 2 * pp + 2
    attn_at = {}
    for (bb, pp), it in attn_iter.items():
        attn_at.setdefault(it, []).append((bb, pp))

    # drains and remaps at block boundaries (sched runs after attn_at,
    # so same-iter drain follows that block's last attn)
    for b, (di, ri) in enumerate(((18, 19), (33, 34), (48, 49))):
        sched[di].append(drain(b, 0))
        sched[di].append(drain(b, 1))
        sched[ri].append(remaps(b, (nc.sync, nc.gpsimd)))

    # final phases: A after block 1 drains, B after block 2 drains
    for k, p in enumerate(final_pieces(0)):
        sched[35 + k].append(p)
    for k, p in enumerate(final_pieces(1)):
        sched[51 + k].append(p)

    # ---------------- main stream ----------------
    for i in range(NB * NT):
        for bb, pp in attn_at.get(i, ()):
            emit_attn(bb, pp)
        for p in sched[i]:
            p()
        if i + 2 < NB * NT:
            emit_scores(i + 2)
        emit_sig(i)

    # ---------------- tail ----------------
    for it in sorted(k for k in attn_at if k >= NB * NT):
        for bb, pp in attn_at[it]:
            emit_attn(bb, pp)
    drain(NB - 1, 0)()
    drain(NB - 1, 1, act=True)()
    remaps(NB - 1, (nc.sync, nc.scalar), chunk_major=True)()
    for p in final_pieces(2):
        p()
    psf_holder["cm"].__exit__(None, None, None)

    if dbg is not None:
        nc.sync.dma_start(dbg["kt2"], kt2)
        nc.sync.dma_start(dbg["qt2"], qt2)
        nc.sync.dma_start(dbg["v2n"], v2n)
        nc.sync.dma_start(dbg["sc00"], sc_tiles[(0, 0)])
        nc.sync.dma_start(dbg["ab00"], ab_tiles[(0, 0)])
        for c in range(NC_F):
            nc.sync.dma_start(dbg[f"ft{c}"], ft_all[:, c * 256:(c + 1) * 256])


def build_bass(replicas: int = 1) -> bass.Bass:
    nc = bacc.Bacc("TRN2", target_bir_lowering=False, debug=False,
                   num_devices=NCORES)
    xt_d = nc.dram_tensor("xt", [D, S], FP32, kind="ExternalInput").ap()
    x8_d = nc.dram_tensor("x8", [128, NB * 4096], FP8,
                          kind="ExternalInput").ap()
    wq_d = nc.dram_tensor("wq", [128, D], FP32, kind="ExternalInput").ap()
    wk_d = nc.dram_tensor("wk", [128, D], FP32, kind="ExternalInput").ap()
    wv_d = nc.dram_tensor("wv", [128, 1024], FP8, kind="ExternalInput").ap()
    wf_d = nc.dram_tensor("wf", [128, NC_F * F], BF16,
                          kind="ExternalInput").ap()
    # compact layout: [ (h,r'), (q,fc)*512 + f' ] -- host unscatters rows
    out_d = nc.dram_tensor("out", [128, 4 * 512], BF16,
                           kind="ExternalOutput").ap()
    dbg = None
    if os.environ.get("V3_DEBUG"):
        dbg = {
            "kt2": nc.dram_tensor("dbg_kt2", [128, S], BF16,
                                  kind="ExternalOutput").ap(),
            "qt2": nc.dram_tensor("dbg_qt2", [128, S], BF16,
                                  kind="ExternalOutput").ap(),
            "v2n": nc.dram_tensor("dbg_v2n", [128, S], FP8,
                                  kind="ExternalOutput").ap(),
            "sc00": nc.dram_tensor("dbg_sc00", [128, 2048], FP8,
                                   kind="ExternalOutput").ap(),
            "ab00": nc.dram_tensor("dbg_ab00", [64, 512], BF16,
                                   kind="ExternalOutput").ap(),
        }
        for c in range(NC_F):
            dbg[f"ft{c}"] = nc.dram_tensor(f"dbg_ft{c}", [128, 256], BF16,
                                           kind="ExternalOutput").ap()
    with tile.TileContext(nc) as tc:
        for _ in range(replicas):
            with ExitStack() as ctx:
                build_kernel(ctx, tc, xt_d, x8_d, wq_d, wk_d, wv_d, wf_d,
                             out_d, dbg)
    nc.finalize()
    return nc


_NC_CACHE = None
_EXEC_CACHE = None


def _get_nc():
    global _NC_CACHE
    if _NC_CACHE is None:
        _NC_CACHE = build_bass()
    return _NC_CACHE


def _get_executor():
    global _EXEC_CACHE
    if _EXEC_CACHE is not None:
        return _EXEC_CACHE
    import concourse.mybir as mybir

    nc = _get_nc()
    install_neuronx_cc_hook()
    partition_name = (nc.partition_id_tensor.name
                      if nc.partition_id_tensor else None)
    in_names, out_names, out_avals = [], [], []
    out_shapes = []
    for alloc in nc.m.functions[0].allocations:
        if not isinstance(alloc, mybir.MemoryLocationSet):
            continue
        name = alloc.memorylocations[0].name
        if alloc.kind == "ExternalInput":
            if name != partition_name:
                in_names.append(name)
        elif alloc.kind == "ExternalOutput":
            shape = tuple(alloc.tensor_shape)
            dtype = mybir.dt.np(alloc.dtype)
            out_names.append(name)
            out_avals.append(jax.core.ShapedArray(shape, dtype))
            out_shapes.append((shape, dtype))
    n_params = len(in_names)
    all_names = list(in_names) + list(out_names)
    if partition_name is not None:
        all_names.append(partition_name)

    def _body(*args):
        operands = list(args)
        if partition_name is not None:
            operands.append(partition_id_tensor())
        outs = _bass_exec_p.bind(
            *operands,
            out_avals=tuple(out_avals),
            in_names=tuple(all_names),
            out_names=tuple(out_names),
            lowering_input_output_aliases=(),
            sim_require_finite=True,
            sim_require_nnan=True,
            nc=nc,
        )
        return tuple(outs)

    devices = jax.devices()[:NCORES]
    mesh = Mesh(np.asarray(devices), ("core",))
    n_args = n_params + len(out_names)
    sharded = jax.jit(shard_map(
        _body, mesh=mesh,
        in_specs=(PartitionSpec("core"),) * n_args,
        out_specs=(PartitionSpec("core"),) * len(out_names),
        check_rep=False))
    _EXEC_CACHE = (sharded, mesh, in_names, out_names, out_shapes)
    return _EXEC_CACHE


def _run_spmd(in_maps):
    sharded, mesh, in_names, out_names, out_shapes = _get_executor()
    sh = NamedSharding(mesh, PartitionSpec("core"))
    args = [np.concatenate([im[name] for im in in_maps], axis=0)
            for name in in_names]
    for shape, dtype in out_shapes:
        args.append(np.zeros((NCORES * shape[0],) + shape[1:], dtype))
    dev_args = [jax.device_put(a, sh) for a in args]
    outs = sharded(*dev_args)
    jax.block_until_ready(outs)
    results = []
    for c in range(NCORES):
        res = {}
        for i, name in enumerate(out_names):
            g = np.asarray(outs[i])
            d0 = g.shape[0] // NCORES
            res[name] = g[c * d0:(c + 1) * d0]
        results.append(res)
    return results


def _layout_w32(w, c):
    """[H, D, P] -> per-core [128, (dchunk8, p2)] fp32 (baseline fp32r)."""
    wl = np.transpose(w[HL * c:HL * (c + 1)], (1, 0, 2)).reshape(D, P2)
    wl = wl.reshape(8, 128, P2).transpose(1, 0, 2).reshape(128, 8 * P2)
    return np.ascontiguousarray(wl, dtype=np.float32)


def _layout_w8(w, c):
    """[H, D, P] -> per-core [128, (c4, i2, m128)] fp8 (DoubleRow pairs)."""
    import ml_dtypes
    wl = np.transpose(w[HL * c:HL * (c + 1)], (1, 0, 2)).reshape(D, P2)
    wl = wl.reshape(4, 2, 128, P2).transpose(2, 0, 1, 3).reshape(128, 1024)
    return np.ascontiguousarray(wl).astype(ml_dtypes.float8_e4m3)


def make_in_maps(x, Qw, Kw, Vw, W_fin):
    import ml_dtypes
    x = np.asarray(x, dtype=np.float32)
    Qw = np.asarray(Qw, dtype=np.float32)
    Kw = np.asarray(Kw, dtype=np.float32)
    Vw = np.asarray(Vw, dtype=np.float32)
    W_fin = np.asarray(W_fin, dtype=np.float32)

    # block-major permutation: pos = b*512 + u2*256 + g*128 + r with
    # u = 4b + 2g + u2, s_orig = 16r + u  (keeps final-linear chunks
    # contiguous per (u2, g) for 3D-balanced remap DMAs)
    pos = np.arange(S)
    b_i, u2_i = pos // 512, (pos // 256) % 2
    g_i, r_i = (pos // 128) % 2, pos % 128
    s_orig = 16 * r_i + 4 * b_i + 2 * g_i + u2_i
    xt = np.ascontiguousarray(x.T)                      # [D, S]
    xtp = np.ascontiguousarray(xt[:, s_orig])
    # fp8 copy in [p, (b, c, i, s)] with d = c*256 + i*128 + p
    xf = xtp.reshape(4, 2, 128, NB, 512).transpose(2, 3, 0, 1, 4).reshape(
        128, NB * 4096)
    xf = np.ascontiguousarray(xf).astype(ml_dtypes.float8_e4m3)
    # wf: natural W_fin contraction chunks [128, c*F + f], bf16
    wf = np.ascontiguousarray(
        W_fin.reshape(NC_F, 128, F).transpose(1, 0, 2).reshape(128, NC_F * F)
    ).astype(ml_dtypes.bfloat16)

    in_maps = []
    for c in range(NCORES):
        in_maps.append({
            "xt": xtp,
            "x8": xf,
            "wq": _layout_w32(Qw, c),
            "wk": _layout_w32(Kw, c),
            "wv": _layout_w8(Vw, c),
            "wf": wf,
        })
    return in_maps


def assemble_out(results, b_fin):
    b_fin = np.asarray(b_fin, dtype=np.float32)
    cores = []
    for c in range(NCORES):
        buf = results[c]["out"].astype(np.float32)      # [128, 2048]
        v = buf.reshape(2, 64, 2, 2, 512)               # [h, r', q, fc, f]
        cores.append(v.transpose(0, 2, 1, 3, 4).reshape(256, F))
    out = np.concatenate(cores, axis=0)
    return (out + b_fin).astype(np.float32)


def kernel(x, Qw, Kw, Vw, W_fin, b_fin):
    in_maps = make_in_maps(x, Qw, Kw, Vw, W_fin)
    results = _run_spmd(in_maps)
    return assemble_out(results, b_fin)
